# revision 48
# baseline (speedup 1.0000x reference)
"""Trainium2 Bass kernel for nn_Attention_3375844294750.

Cross-attention (q from x, k/v from context) with key mask, 8 heads, d=64.
  B=4, N=M=2048, query_dim=context_dim=512, inner=512.

Sharding: 8 NeuronCores = (batch b = core//2) x (query-half = core%2).
Each core computes attention for its 1024 queries over its batch's keys.
No collectives needed (outputs are disjoint).

Key compaction: masked keys contribute exactly 0 to masked softmax, so the
CPU glue gathers only the unmasked keys (~50% of 2048) per batch, padded
to a multiple of 128; padding slots are killed by the exp bias.

Structure (nb outer, head-pair inner):
  for nb in (0, 1):            # query half of this core's 1024 queries
    for p in (0..3):           # head pair
      stream over m-tiles: S^T -> exp (fp8 out) -> PV
PV uses fp8e4 DoubleRow matmuls over m-tile PAIRS (K=256 per instruction):
the ACT exp writes P^T directly as fp8 into a [tile,head,n] layout whose
pair stride is 16B-aligned, and V is stored fp8 with a ones column so the
softmax denominator rides along as PSUM row 64.  Scores stay bf16 (K=64
contraction gets no DoubleRow win).  The out-projection accumulates all
four head-pairs in one PSUM tile via a chained matmul group (one DVE
bias-add per 128-query block instead of one add per pair), with nb0's
out-proj dripped into nb1's PE slack so its output DMA overlaps compute.
The last subblock keeps the latency-optimized drain: ACT does the PSUM
reads, the then-idle PE broadcasts the denominator rows, and pair 3 of
the out-projection takes the second head half straight from the
normalize result via a split-K accumulation.

Startup: inputs load as fine-grained multi-dim DMAs over the two DGE
queues ordered by first use (pair-0 weight columns, first query half,
first context tiles), so the score/exp stream starts a few us in;
throwaway matmuls ramp the PE clock and a throwaway exp pulls the
activation-table load off the critical path.
"""
import os
import sys

for _p in ("/opt/trn_rl_repo", "/root/.axon_site/_ro/trn_rl_repo"):
    if os.path.isdir(_p) and _p not in sys.path:
        sys.path.insert(0, _p)
        break

import numpy as np
import ml_dtypes

B, N, M = 4, 2048, 2048
QD = 512          # query_dim == context_dim
H, D = 8, 64
INNER = H * D     # 512
SCALE = D ** -0.5
NCORE = N // 2    # queries per core = 1024
P = 128
NBLK = 512        # n-block (one PSUM bank per matmul)
MASK_NEG = -1e30

_CACHE = {}


def _build_nc(nmt):
    """Build + compile the SPMD program for nmt m-tiles (m_pad = 128*nmt)."""
    import concourse.mybir as mybir
    from concourse import bacc
    from concourse.tile import TileContext
    import concourse.bass as bass

    mpad = nmt * P
    ntp = nmt // 2           # full m-tile pairs for DoubleRow PV
    leftover = nmt % 2       # odd trailing m-tile
    dt = mybir.dt
    nc = bacc.Bacc("TRN2", target_bir_lowering=False, debug=False, num_devices=8)

    xT_d = nc.declare_dram_parameter("xT", [4, P, NCORE], dt.bfloat16, isOutput=False)
    ctxT_d = nc.declare_dram_parameter("ctxT", [4, P, mpad], dt.bfloat16, isOutput=False)
    wq_d = nc.declare_dram_parameter("wq", [4, P, INNER], dt.bfloat16, isOutput=False)
    wk_d = nc.declare_dram_parameter("wk", [4, P, INNER], dt.bfloat16, isOutput=False)
    wv_d = nc.declare_dram_parameter("wv", [4, P, INNER], dt.bfloat16, isOutput=False)
    wo_d = nc.declare_dram_parameter("wo", [4, P, QD], dt.bfloat16, isOutput=False)
    bo_d = nc.declare_dram_parameter("bo", [1, QD], dt.float32, isOutput=False)
    mb_d = nc.declare_dram_parameter("mb", [P, nmt], dt.float32, isOutput=False)
    out_d = nc.declare_dram_parameter("out", [NCORE, QD], dt.bfloat16, isOutput=True)

    f32 = dt.float32
    bf16 = dt.bfloat16
    fp8 = dt.float8e4
    EXP = mybir.ActivationFunctionType.Exp
    CPY = mybir.ActivationFunctionType.Copy
    DR = mybir.MatmulPerfMode.DoubleRow

    # v tile free-dim stride: D+1 (values + ones row), padded so the
    # DoubleRow pair stride (H*VS bytes, fp8) is a multiple of 16
    VS = 80

    with TileContext(nc) as tc:
        from contextlib import ExitStack

        with ExitStack() as ctx:
            const = ctx.enter_context(tc.tile_pool(name="const", bufs=1))

            # ---- persistent SBUF tensors ----
            wq_sb = const.tile([P, 4, INNER], bf16, tag="wq")
            xT_sb = const.tile([P, 4, NCORE], bf16, tag="xT")
            wk_sb = const.tile([P, 4, INNER], bf16, tag="wk")
            ctxT_sb = const.tile([P, 4, mpad], bf16, tag="cT")
            wv_sb = const.tile([P, 4, INNER], bf16, tag="wv")
            wo_sb = const.tile([P, 4, QD], bf16, tag="wo")
            bo_bc = const.tile([P, QD], f32, tag="bo")
            mb_sb = const.tile([P, nmt], f32, tag="mb")
            wm_sb = const.tile([P, NBLK], bf16, tag="wm")

            ones_sb = const.tile([P, D], bf16, tag="ones")
            dm_sb = const.tile([P, 8], bf16, tag="dm")
            wo3b = const.tile([D, QD], bf16, tag="wo3b")
            qT_sb = const.tile([P, 4, NCORE], bf16, tag="qT")
            kT_sb = const.tile([P, 4, mpad], bf16, tag="kT")
            # fp8 V with ones column, [tilepair][tile-in-pair][head][VS]
            nvt = ntp + leftover
            v8_sb = const.tile([P, nvt, 2, H, VS], fp8, tag="v8")
            o_sb = const.tile([P, 4, NCORE], bf16, tag="oT")
            fin_sb = const.tile([P, NCORE // P, QD], bf16, tag="fin")

            # warm-up weights for the PE clock ramp (never read by output)
            nc.vector.memset(wm_sb[:], 1.0)
            nc.vector.memset(ones_sb[:], 1.0)

            # ---- input loads: fine-grained multi-dim DMAs over two issue
            # queues, ordered by first use
            def dbox(t, s0, ns, p0, np_, c0, nc_):
                a = t.ap()
                S, Pp, C = a.ap[0][0], a.ap[1][0], a.ap[2][0]
                return bass.AP(
                    tensor=a.tensor,
                    offset=a.offset + s0 * S + p0 * Pp + c0 * C,
                    ap=[[Pp, np_], [S, ns], [C, nc_]])

            # m-tile column chunks for the k projection (also the DMA
            # split points for ctxT): first the two tiles the stream
            # opens with, then the rest
            c_t01 = min(2 * P, mpad)
            mchunks = [(0, c_t01)]
            off = c_t01
            while off < mpad:
                w = min(NBLK, mpad - off)
                mchunks.append((off, w))
                off += w

            # one queue, strict FIFO, ordered by first use; ~250KB+ per
            # DMA so the wire (not the ~625ns/DMA issue rate) is the
            # limiter.  The first query half comes in two kq-halves so
            # the first projection matmuls start while the wire runs.
            nc.sync.dma_start(out=xT_sb[:, 0:2, 0:NBLK],
                              in_=dbox(xT_d, 0, 2, 0, P, 0, NBLK))
            nc.sync.dma_start(out=wq_sb[:, :, 0:P],
                              in_=dbox(wq_d, 0, 4, 0, P, 0, P))
            nc.sync.dma_start(out=xT_sb[:, 2:4, 0:NBLK],
                              in_=dbox(xT_d, 2, 2, 0, P, 0, NBLK))
            nc.sync.dma_start(out=ctxT_sb[:, :, 0:c_t01],
                              in_=dbox(ctxT_d, 0, 4, 0, P, 0, c_t01))
            nc.sync.dma_start(out=wk_sb[:, :, 0:P],
                              in_=dbox(wk_d, 0, 4, 0, P, 0, P))
            nc.sync.dma_start(out=mb_sb[:], in_=mb_d[:])

            # remaining inputs, same queue (strict FIFO keeps the wire in
            # priority order), ordered by first use
            for coff, cw in mchunks[1:]:
                nc.sync.dma_start(out=ctxT_sb[:, :, coff:coff + cw],
                                  in_=dbox(ctxT_d, 0, 4, 0, P, coff, cw))
            nc.sync.dma_start(out=wv_sb[:],
                              in_=dbox(wv_d, 0, 4, 0, P, 0, INNER))
            nc.sync.dma_start(out=wq_sb[:, :, P:INNER],
                              in_=dbox(wq_d, 0, 4, 0, P, P, INNER - P))
            nc.sync.dma_start(out=wk_sb[:, :, P:INNER],
                              in_=dbox(wk_d, 0, 4, 0, P, P, INNER - P))
            nc.sync.dma_start(out=xT_sb[:, :, NBLK:NCORE],
                              in_=dbox(xT_d, 0, 4, 0, P, NBLK, NCORE - NBLK))
            nc.sync.dma_start(out=wo_sb[:],
                              in_=dbox(wo_d, 0, 4, 0, P, 0, QD))
            nc.sync.dma_start(out=wo3b[:], in_=dbox(wo_d, 3, 1, D, D, 0, QD))
            bo_src = bass.AP(tensor=bo_d.ap().tensor, offset=bo_d.ap().offset,
                             ap=[[0, P]] + bo_d.ap().ap[1:])
            nc.sync.dma_start(out=bo_bc[:], in_=bo_src)

            # ones columns for the softmax denominator (v copies leave them)
            nc.vector.memset(v8_sb[:, :, :, :, D:D + 1], 1.0)

            with tc.tile_pool(name="aux", bufs=2, space="PSUM") as aux, \
                 tc.tile_pool(name="sps", bufs=2, space="PSUM") as sps, \
                 tc.tile_pool(name="ops", bufs=1, space="PSUM") as ops, \
                 tc.tile_pool(name="ppool", bufs=5) as ppool, \
                 tc.tile_pool(name="raw", bufs=4) as rawp, \
                 tc.tile_pool(name="bcp", bufs=2) as bcp, \
                 tc.tile_pool(name="dscr", bufs=4, space="DRAM") as dscr:

                # a few throwaway matmuls ramp the PE p-state while the
                # first input tiles are still in flight, and a throwaway
                # exp pulls the activation table load off the critical path
                for _ in range(4):
                    pw = aux.tile([P, NBLK], f32, tag="aux", name="warm")
                    nc.tensor.matmul(pw[:], lhsT=wm_sb[:, 0:P],
                                     rhs=wm_sb[:], start=True, stop=True)
                nc.scalar.activation(out=dm_sb[0:1, :], in_=wm_sb[0:1, 0:8],
                                     func=EXP, scale=1.0)

                def v_unit(mt):
                    def f():
                        ps = aux.tile([P, INNER], f32, tag="aux", name="psv")
                        for kq in range(4):
                            nc.tensor.matmul(
                                ps[:],
                                lhsT=ctxT_sb[:, kq, mt * P:(mt + 1) * P],
                                rhs=wv_sb[:, kq, :],
                                start=(kq == 0), stop=(kq == 3),
                            )
                        psh = ps.rearrange("p (h d) -> p h d", h=H)
                        nc.vector.tensor_copy(
                            v8_sb[:, mt // 2, mt % 2, :, 0:D], psh[:])
                    return f

                def proj_unit_q(mi, nh):
                    def f():
                        ps = aux.tile([P, NBLK], f32, tag="aux", name="psq")
                        for kq in range(4):
                            nc.tensor.matmul(
                                ps[:],
                                lhsT=wq_sb[:, kq, mi * P:(mi + 1) * P],
                                rhs=xT_sb[:, kq, nh * NBLK:(nh + 1) * NBLK],
                                start=(kq == 0), stop=(kq == 3),
                            )
                        nc.vector.tensor_copy(
                            qT_sb[:, mi, nh * NBLK:(nh + 1) * NBLK], ps[:])
                    return f

                def proj_unit_k(mi, off, w):
                    def f():
                        ps = aux.tile([P, NBLK], f32, tag="aux", name="psk")
                        for kq in range(4):
                            nc.tensor.matmul(
                                ps[:, 0:w],
                                lhsT=wk_sb[:, kq, mi * P:(mi + 1) * P],
                                rhs=ctxT_sb[:, kq, off:off + w],
                                start=(kq == 0), stop=(kq == 3),
                            )
                        nc.vector.tensor_copy(
                            kT_sb[:, mi, off:off + w], ps[:, 0:w])
                    return f

                def fin_unit4(nt):
                    # out-proj for 128 queries: all four head-pairs chained
                    # into one PSUM accumulation, one bias-add, one DMA out
                    def f():
                        ps = aux.tile([P, NBLK], f32, tag="aux", name="psf")
                        for pp in range(4):
                            nc.tensor.matmul(
                                ps[:, 0:QD],
                                lhsT=o_sb[:, pp, nt * P:(nt + 1) * P],
                                rhs=wo_sb[:, pp, :],
                                start=(pp == 0), stop=(pp == 3),
                            )
                        nc.vector.tensor_add(
                            fin_sb[:, nt, :], ps[:, 0:QD], bo_bc[:])
                        nc.sync.dma_start(
                            out=out_d[nt * P:(nt + 1) * P, :],
                            in_=fin_sb[:, nt, :])
                    return f

                def fin_unit3(nt):
                    # out-proj partial for the last nb: pairs 0-2 chained,
                    # bias-added into fin_sb; pair 3 lands in the tail
                    def f():
                        ps = aux.tile([P, NBLK], f32, tag="aux", name="psf")
                        for pp in range(3):
                            nc.tensor.matmul(
                                ps[:, 0:QD],
                                lhsT=o_sb[:, pp, nt * P:(nt + 1) * P],
                                rhs=wo_sb[:, pp, :],
                                start=(pp == 0), stop=(pp == 2),
                            )
                        nc.vector.tensor_add(
                            fin_sb[:, nt, :], ps[:, 0:QD], bo_bc[:])
                    return f

                def fin_tail_pair(nt0, tb, c0):
                    # pair 3 of the final half for two 128-query blocks,
                    # straight from the normalize result via split-K (no
                    # shift DMA), matmuls hoisted ahead of the adds so
                    # add/DMA pipeline behind the PE
                    pss = []
                    for nt in (nt0, nt0 + 1):
                        tof = (nt - 4) * P - c0
                        ps = aux.tile([P, NBLK], f32, tag="aux", name="psf")
                        nc.tensor.matmul(
                            ps[:, 0:QD],
                            lhsT=o_sb[0:D, 3, nt * P:(nt + 1) * P],
                            rhs=wo_sb[0:D, 3, :],
                            start=True, stop=False,
                        )
                        nc.tensor.matmul(
                            ps[:, 0:QD],
                            lhsT=tb[0:D, tof:tof + P],
                            rhs=wo3b[:],
                            start=False, stop=True,
                        )
                        pss.append(ps)
                    for nt, ps in zip((nt0, nt0 + 1), pss):
                        nc.vector.tensor_add(
                            fin_sb[:, nt, :], ps[:, 0:QD], fin_sb[:, nt, :])
                        nc.sync.dma_start(
                            out=out_d[nt * P:(nt + 1) * P, :],
                            in_=fin_sb[:, nt, :])

                def fin_tail_a(nt):
                    # pair 3 of the first half: plain K=128 matmul off the
                    # shifted o_sb, dripped under the second half's stream
                    def f():
                        ps = aux.tile([P, NBLK], f32, tag="aux", name="psf")
                        nc.tensor.matmul(
                            ps[:, 0:QD],
                            lhsT=o_sb[:, 3, nt * P:(nt + 1) * P],
                            rhs=wo_sb[:, 3, :],
                            start=True, stop=True,
                        )
                        nc.vector.tensor_add(
                            fin_sb[:, nt, :], ps[:, 0:QD], fin_sb[:, nt, :])
                        nc.sync.dma_start(
                            out=out_d[nt * P:(nt + 1) * P, :],
                            in_=fin_sb[:, nt, :])
                    return f

                # Aux-work schedule. Every subblock has:
                #   prep[nb][p]: units emitted inline before its first
                #     score (data its own stream needs that could not be
                #     dripped earlier, e.g. DMA not yet landed)
                #   drip[nb][p]: units popped into the stream, at most
                #     `pops` per m-tile, emission order tracking expected
                #     DMA arrival so no waiting unit clogs the in-order
                #     PE queue ahead of ready work.  Anything left is
                #     flushed at the subblock end (before the drain pvs).
                def kprep(p, tail=False):
                    chs = mchunks[1:] if tail else mchunks
                    return [proj_unit_k(p, coff, cw) for coff, cw in chs]

                def stagger(start, units, step=1):
                    return [(start + i * step, u) for i, u in enumerate(units)]

                nv0 = min(7, nmt)   # v tiles dripped in p0; rest go to p1
                prep = {(0, 0): [proj_unit_q(0, 0), proj_unit_k(0, *mchunks[0])]}
                drip = {
                    # p0: its own late k chunks first (ctxT lands just in
                    # time), then the v tiles once wv has landed, then
                    # pair 1's projections (weights land last)
                    (0, 0): stagger(1, kprep(0, tail=True), 2)
                            + [(2 + mt // 2, v_unit(mt))
                               for mt in range(nv0)]
                            + [(6, proj_unit_q(1, 0)),
                               (7, proj_unit_k(1, *mchunks[0]))],
                    (0, 1): [(0, v_unit(mt)) for mt in range(nv0, nmt)]
                            + stagger(0, kprep(1, tail=True))
                            + stagger(3, [proj_unit_q(2, 0)] + kprep(2)),
                    (0, 2): stagger(0, [proj_unit_q(3, 0)] + kprep(3))
                            + [(4, proj_unit_q(0, 1))],
                    (0, 3): [],
                    (1, 0): [(0, proj_unit_q(1, 1))]
                            + [(nt + 3, fin_unit4(nt)) for nt in range(4)],
                    (1, 1): [(0, proj_unit_q(2, 1))],
                    (1, 2): [(0, proj_unit_q(3, 1))],
                    (1, 3, "b"): [(nt - 2, fin_unit3(nt))
                                  for nt in range(4, 8)],
                }

                # each subblock defers its last pv pairs + normalize into
                # the next subblock's drip slots (the next stream's early
                # m-tiles are PE-light), so pair boundaries don't stall
                # the exp stream on a serial drain
                # subblock sequence; the last subblock takes the "b"
                # latency-optimized drain
                seq = ([(0, pq, 0, NBLK, "n") for pq in range(4)]
                       + [(1, pq, 0, NBLK, "n") for pq in range(3)]
                       + [(1, 3, 0, NBLK, "b")])

                carry = []
                final = []
                for si, (nb, p, c0, cw, kind) in enumerate(seq):
                    if si == 0:
                        for u in prep.get((0, 0), []):
                            u()
                    pending = sorted(drip.get((nb, p, kind),
                                              drip.get((nb, p), []))
                                     + carry, key=lambda e: e[0])
                    carry = []

                    nsl = slice(nb * NBLK + c0, nb * NBLK + c0 + cw)
                    oa = ops.tile([P, NBLK], f32, tag="oa")
                    ob = ops.tile([P, NBLK], f32, tag="ob")
                    pts = {}
                    npv = [0]

                    def pv(tp, p=p, oa=oa, ob=ob, pts=pts, npv=npv, cw=cw):
                        # one DoubleRow matmul per head covers both
                        # m-tiles of the pair (K=256); an odd trailing
                        # tile (tp == ntp) uses a normal fp8 matmul
                        pt = pts.pop(tp)
                        first = npv[0] == 0
                        last = tp == (ntp + leftover - 1)
                        npv[0] += 1
                        if tp < ntp:
                            for h, ot in ((0, oa), (1, ob)):
                                nc.tensor.matmul(
                                    ot[0:D + 1, 0:cw],
                                    lhsT=v8_sb[:, tp, :, 2 * p + h, 0:D + 1],
                                    rhs=pt[:, :, h, 0:cw],
                                    start=first, stop=last,
                                    perf_mode=DR,
                                )
                        else:
                            for h, ot in ((0, oa), (1, ob)):
                                nc.tensor.matmul(
                                    ot[0:D + 1, 0:cw],
                                    lhsT=v8_sb[:, tp, 0, 2 * p + h, 0:D + 1],
                                    rhs=pt[:, 0, h, 0:cw],
                                    start=first, stop=last,
                                )

                    for mt in range(nmt):
                        tp, ti = mt // 2, mt % 2
                        if ti == 0:
                            pts[tp] = ppool.tile([P, 2, 2, NBLK], fp8,
                                                 tag="pt", name="pt")
                        sp = sps.tile([P, 2 * NBLK], f32, tag="s")
                        msl = slice(mt * P, (mt + 1) * P)
                        nc.tensor.matmul(
                            sp[:, 0:cw],
                            lhsT=kT_sb[0:64, p, msl],
                            rhs=qT_sb[0:64, p, nsl],
                            start=True, stop=True,
                        )
                        nc.tensor.matmul(
                            sp[:, cw:2 * cw],
                            lhsT=kT_sb[64:128, p, msl],
                            rhs=qT_sb[64:128, p, nsl],
                            start=True, stop=True,
                        )
                        # exp writes P^T as fp8 [head, n] for this tile
                        sph = sp[:, 0:2 * cw].rearrange(
                            "q (h n) -> q h n", h=2)
                        nc.scalar.activation(
                            out=pts[tp][:, ti, :, 0:cw], in_=sph[:],
                            func=EXP,
                            bias=mb_sb[:, mt:mt + 1], scale=SCALE,
                        )
                        # PV lags the exp stream so the in-order PE
                        # queue never stalls on the ACT stream (deeper
                        # lag in the very first subblock, where the v
                        # tiles are still being produced)
                        lag = 4 if si == 0 else 3
                        if mt >= lag and (mt - lag) % 2 == 1:
                            pv((mt - lag) // 2)
                        pops = 0
                        while (pending and pending[0][0] <= mt
                               and pops < 2):
                            pending.pop(0)[1]()
                            pops += 1
                    for _, u in pending:
                        u()

                    def normalize(p=p, nsl=nsl, oa=oa, ob=ob, cw=cw):
                        # normalize: bounce the raw denominator rows
                        # through DRAM for the partition broadcast,
                        # reciprocal out of place, then multiply
                        rawa = rawp.tile([P, NBLK], f32, tag="rawa",
                                         name="rawa")
                        rawb = rawp.tile([P, NBLK], f32, tag="rawb",
                                         name="rawb")
                        tb = rawp.tile([D, NBLK], bf16, tag="tb",
                                       name="tb")
                        bcb = bcp.tile([D, 2, NBLK], f32, tag="bcb",
                                       name="bcb")
                        rcb = bcp.tile([D, 2, NBLK], f32, tag="rcb",
                                       name="rcb")
                        scr = dscr.tile([2, NBLK], f32, tag="scr",
                                        name="scr")
                        nc.vector.tensor_copy(rawa[0:D + 1, 0:cw],
                                              oa[0:D + 1, 0:cw])
                        nc.vector.tensor_copy(rawb[0:D + 1, 0:cw],
                                              ob[0:D + 1, 0:cw])
                        for i, raw in ((0, rawa), (1, rawb)):
                            nc.sync.dma_start(out=scr[i:i + 1, 0:cw],
                                              in_=raw[D:D + 1, 0:cw])
                            src = scr[i:i + 1, 0:cw]
                            bsrc = bass.AP(tensor=src.tensor,
                                           offset=src.offset,
                                           ap=[[0, D]] + src.ap[1:])
                            nc.sync.dma_start(out=rcb[0:D, i, 0:cw],
                                              in_=bsrc)
                        nc.vector.reciprocal_approx_fast(
                            out=bcb[0:D, :, 0:cw], in_=rcb[0:D, :, 0:cw])
                        nc.vector.tensor_mul(
                            o_sb[0:D, p, nsl], rawa[0:D, 0:cw],
                            bcb[0:D, 0, 0:cw])
                        nc.vector.tensor_mul(
                            tb[0:D, 0:cw], rawb[0:D, 0:cw],
                            bcb[0:D, 1, 0:cw])
                        nc.sync.dma_start(out=o_sb[D:P, p, nsl],
                                          in_=tb[0:D, 0:cw])

                    if kind != "b":
                        # the deferred pvs and normalize must all pop
                        # before the next subblock's own first pv
                        # (which reuses the single-buffered oa/ob)
                        rem = sorted(pts)
                        carry = [(i // 2, (lambda tp=tp, pv=pv: pv(tp)))
                                 for i, tp in enumerate(rem)]
                        carry.append(((len(rem) + 1) // 2, normalize))
                    else:
                        # final half: latency-optimized drain. ACT (its
                        # exp stream just ended) reads only the
                        # denominator rows, the idle PE broadcasts them,
                        # the muls read the PV accumulators straight from
                        # PSUM, and pair 3 of the out-proj takes the
                        # second head half straight from tb
                        for tp in sorted(pts):
                            pv(tp)
                        tb = rawp.tile([D, NBLK], bf16, tag="tb",
                                       name="tb")
                        bcb = bcp.tile([D, 2, NBLK], f32, tag="bcb",
                                       name="bcb")
                        den = bcp.tile([P, 2, NBLK], bf16, tag="den")
                        nc.scalar.activation(out=den[D:D + 1, 0, 0:cw],
                                             in_=oa[D:D + 1, 0:cw], func=CPY)
                        nc.scalar.activation(out=den[D:D + 1, 1, 0:cw],
                                             in_=ob[D:D + 1, 0:cw], func=CPY)
                        bca = aux.tile([P, NBLK], f32, tag="aux",
                                       name="bca")
                        bcq = aux.tile([P, NBLK], f32, tag="aux",
                                       name="bcq")
                        nc.tensor.matmul(
                            bca[0:D, 0:cw], lhsT=ones_sb[D:D + 1, :],
                            rhs=den[D:D + 1, 0, 0:cw],
                            start=True, stop=True)
                        nc.tensor.matmul(
                            bcq[0:D, 0:cw], lhsT=ones_sb[D:D + 1, :],
                            rhs=den[D:D + 1, 1, 0:cw],
                            start=True, stop=True)
                        HB = cw // 2
                        nc.vector.reciprocal_approx_fast(
                            out=bcb[0:D, 0, 0:cw], in_=bca[0:D, 0:cw])
                        for hh in range(2):
                            hsl = slice(hh * HB, (hh + 1) * HB)
                            osl = slice(nb * NBLK + c0 + hh * HB,
                                        nb * NBLK + c0 + (hh + 1) * HB)
                            nc.vector.tensor_mul(
                                o_sb[0:D, 3, osl], oa[0:D, hsl],
                                bcb[0:D, 0, hsl])
                            if hh == 0:
                                nc.vector.reciprocal_approx_fast(
                                    out=bcb[0:D, 1, 0:cw],
                                    in_=bcq[0:D, 0:cw])
                            nc.vector.tensor_mul(
                                tb[0:D, hsl], ob[0:D, hsl],
                                bcb[0:D, 1, hsl])
                            final.append(
                                lambda nt0=4 + (c0 + hh * HB) // P, tb=tb:
                                fin_tail_pair(nt0, tb, c0))
                # drain: the final half's out-proj tail
                for f in final:
                    f()

    nc.compile()
    return nc


def get_nc(nmt=None):
    if nmt is None:
        nmt = _CACHE.get("last_nmt", M // P)
    if ("nc", nmt) not in _CACHE:
        _CACHE[("nc", nmt)] = _build_nc(nmt)
    _CACHE["last_nmt"] = nmt
    return _CACHE[("nc", nmt)]


def make_in_maps(x, context, mask, Wq, Wkv, Wo, bo):
    """CPU glue: shard, transpose, cast, and compact keys by mask."""
    bf = ml_dtypes.bfloat16
    Wk = np.ascontiguousarray(Wkv[:, :INNER]).astype(bf)
    Wv = np.ascontiguousarray(Wkv[:, INNER:]).astype(bf)
    Wq_b = np.ascontiguousarray(Wq).astype(bf)
    Wo_b = np.ascontiguousarray(Wo).astype(bf)
    bo_f = np.ascontiguousarray(bo, dtype=np.float32).reshape(1, QD)

    idxs = [np.where(mask[b])[0] for b in range(B)]
    maxc = max(1, max(len(i) for i in idxs))
    nmt = (maxc + P - 1) // P
    mpad = nmt * P

    in_maps = []
    for c in range(8):
        b, s = c // 2, c % 2
        idx = idxs[b]
        cnt = len(idx)
        ctx_c = np.zeros((mpad, QD), dtype=np.float32)
        ctx_c[:cnt] = context[b][idx]
        mb = np.full(mpad, MASK_NEG, dtype=np.float32)
        mb[:cnt] = 0.0
        xT = np.ascontiguousarray(
            x[b, s * NCORE:(s + 1) * NCORE, :].T).astype(bf)
        ctxT = np.ascontiguousarray(ctx_c.T).astype(bf)
        mbt = np.ascontiguousarray(mb.reshape(nmt, P).T)
        in_maps.append({
            "xT": xT.reshape(4, P, NCORE),
            "ctxT": ctxT.reshape(4, P, mpad),
            "wq": Wq_b.reshape(4, P, INNER),
            "wk": Wk.reshape(4, P, INNER),
            "wv": Wv.reshape(4, P, INNER),
            "wo": Wo_b.reshape(4, P, QD),
            "bo": bo_f, "mb": mbt,
        })
    return in_maps, nmt


def assemble(results):
    out = np.empty((B, N, QD), dtype=np.float32)
    for c in range(8):
        b, s = c // 2, c % 2
        out[b, s * NCORE:(s + 1) * NCORE, :] = np.asarray(
            results[c]["out"], dtype=np.float32)
    return out


def kernel(x, context, mask, Wq, Wkv, Wo, bo):
    from concourse.bass_utils import run_bass_kernel_spmd

    x = np.asarray(x, dtype=np.float32)
    context = np.asarray(context, dtype=np.float32)
    mask = np.asarray(mask)
    in_maps, nmt = make_in_maps(x, context, mask,
                                np.asarray(Wq, dtype=np.float32),
                                np.asarray(Wkv, dtype=np.float32),
                                np.asarray(Wo, dtype=np.float32),
                                np.asarray(bo, dtype=np.float32))
    nc = get_nc(nmt)
    res = run_bass_kernel_spmd(nc, in_maps, list(range(8)))
    return assemble(res.results)


# revision 60
# speedup vs baseline: 1.0023x; 1.0023x over previous
"""Trainium2 Bass kernel for nn_Attention_3375844294750.

Cross-attention (q from x, k/v from context) with key mask, 8 heads, d=64.
  B=4, N=M=2048, query_dim=context_dim=512, inner=512.

Sharding: 8 NeuronCores = (batch b = core//2) x (query-half = core%2).
Each core computes attention for its 1024 queries over its batch's keys.
No collectives needed (outputs are disjoint).

Key compaction: masked keys contribute exactly 0 to masked softmax, so the
CPU glue gathers only the unmasked keys (~50% of 2048) per batch, padded
to a multiple of 128; padding slots are killed by the exp bias.

Structure (nb outer, head-pair inner):
  for nb in (0, 1):            # query half of this core's 1024 queries
    for p in (0..3):           # head pair
      stream over m-tiles: S^T -> exp (fp8 out) -> PV
PV uses fp8e4 DoubleRow matmuls over m-tile PAIRS (K=256 per
instruction, ~1.5-2x the bf16 matmul rate): the ACT exp writes P^T
directly as fp8e4 into a [tile,head,n] layout whose pair stride is
16B-aligned, and V is stored fp8 with a ones column so the softmax
denominator rides along as PSUM row 64.  The unnormalized exp values
(logits bounded, no max subtraction) sit in e4m3's sweet range, and the
denominator is computed from the same quantized P, cancelling the
common-mode quantization error.  Scores stay bf16 (a K=64 contraction
gets no DoubleRow win on hardware).  The out-projection accumulates all
four head-pairs in one PSUM tile via a chained matmul group (one DVE
bias-add per 128-query block instead of one add per pair); nb0's
out-proj + output DMA drip into nb1's PE slack.

Scheduling: each subblock's projection/out-proj prep is dripped into
earlier subblocks' PE slack (at most two units per m-tile, emission
order tracking expected DMA arrival so waiting matmuls never clog the
in-order PE queue), and each subblock's last PV pairs + normalize are
deferred into the NEXT subblock's early, PE-light m-tiles so pair
boundaries never stall the exp stream on a serial drain.  The
normalize broadcasts denominator rows across partitions via a DRAM
bounce hidden under the next subblock (gpsimd partition_broadcast
passes CoreSim but returns garbage on HW).  The last subblock keeps a
latency-optimized drain: ACT copies only the denominator rows, the
then-idle PE broadcasts them, the muls read the PV accumulators
straight from PSUM, and pair 3 of the out-projection takes the second
head half straight from the normalize result via a split-K
accumulation.  Output is stored bf16 to halve the final DMA.

Startup: inputs load on one strict-FIFO DGE queue ordered by first use
at ~250KB+ granularity (so the wire, not the ~650ns/DMA issue rate, is
the limiter), the first query half split in two kq-halves and pair 0's
first k-chunk split per tile so the first score waits on as little as
possible; throwaway matmuls ramp the PE clock and a throwaway exp pulls
the activation-table load off the critical path.
"""
import os
import sys

for _p in ("/opt/trn_rl_repo", "/root/.axon_site/_ro/trn_rl_repo"):
    if os.path.isdir(_p) and _p not in sys.path:
        sys.path.insert(0, _p)
        break

import numpy as np
import ml_dtypes

B, N, M = 4, 2048, 2048
QD = 512          # query_dim == context_dim
H, D = 8, 64
INNER = H * D     # 512
SCALE = D ** -0.5
NCORE = N // 2    # queries per core = 1024
P = 128
NBLK = 512        # n-block (one PSUM bank per matmul)
MASK_NEG = -1e30

_CACHE = {}


def _build_nc(nmt):
    """Build + compile the SPMD program for nmt m-tiles (m_pad = 128*nmt)."""
    import concourse.mybir as mybir
    from concourse import bacc
    from concourse.tile import TileContext
    import concourse.bass as bass

    mpad = nmt * P
    ntp = nmt // 2           # full m-tile pairs for DoubleRow PV
    leftover = nmt % 2       # odd trailing m-tile
    dt = mybir.dt
    nc = bacc.Bacc("TRN2", target_bir_lowering=False, debug=False, num_devices=8)

    xT_d = nc.declare_dram_parameter("xT", [4, P, NCORE], dt.bfloat16, isOutput=False)
    ctxT_d = nc.declare_dram_parameter("ctxT", [4, P, mpad], dt.bfloat16, isOutput=False)
    wq_d = nc.declare_dram_parameter("wq", [4, P, INNER], dt.bfloat16, isOutput=False)
    wk_d = nc.declare_dram_parameter("wk", [4, P, INNER], dt.bfloat16, isOutput=False)
    wv_d = nc.declare_dram_parameter("wv", [4, P, INNER], dt.bfloat16, isOutput=False)
    wo_d = nc.declare_dram_parameter("wo", [4, P, QD], dt.bfloat16, isOutput=False)
    bo_d = nc.declare_dram_parameter("bo", [1, QD], dt.float32, isOutput=False)
    mb_d = nc.declare_dram_parameter("mb", [P, nmt], dt.float32, isOutput=False)
    out_d = nc.declare_dram_parameter("out", [NCORE, QD], dt.bfloat16, isOutput=True)

    f32 = dt.float32
    bf16 = dt.bfloat16
    fp8 = dt.float8e4
    EXP = mybir.ActivationFunctionType.Exp
    CPY = mybir.ActivationFunctionType.Copy
    DR = mybir.MatmulPerfMode.DoubleRow

    # v tile free-dim stride: D+1 (values + ones row), padded so the
    # DoubleRow pair stride (H*VS bytes, fp8) is a multiple of 16
    VS = 80

    with TileContext(nc) as tc:
        from contextlib import ExitStack

        with ExitStack() as ctx:
            const = ctx.enter_context(tc.tile_pool(name="const", bufs=1))

            # ---- persistent SBUF tensors ----
            wq_sb = const.tile([P, 4, INNER], bf16, tag="wq")
            xT_sb = const.tile([P, 4, NCORE], bf16, tag="xT")
            wk_sb = const.tile([P, 4, INNER], bf16, tag="wk")
            ctxT_sb = const.tile([P, 4, mpad], bf16, tag="cT")
            wv_sb = const.tile([P, 4, INNER], bf16, tag="wv")
            wo_sb = const.tile([P, 4, QD], bf16, tag="wo")
            bo_bc = const.tile([P, QD], f32, tag="bo")
            mb_sb = const.tile([P, nmt], f32, tag="mb")
            wm_sb = const.tile([P, NBLK], bf16, tag="wm")

            ones_sb = const.tile([P, D], bf16, tag="ones")
            dm_sb = const.tile([P, 8], bf16, tag="dm")
            wo3b = const.tile([D, QD], bf16, tag="wo3b")
            qT_sb = const.tile([P, 4, NCORE], bf16, tag="qT")
            kT_sb = const.tile([P, 4, mpad], bf16, tag="kT")
            # fp8 V with ones column, [tilepair][tile-in-pair][head][VS]
            nvt = ntp + leftover
            v8_sb = const.tile([P, nvt, 2, H, VS], fp8, tag="v8")
            o_sb = const.tile([P, 4, NCORE], bf16, tag="oT")
            fin_sb = const.tile([P, NCORE // P, QD], bf16, tag="fin")

            # warm-up weights for the PE clock ramp (never read by output)
            nc.vector.memset(wm_sb[:], 1.0)
            nc.vector.memset(ones_sb[:], 1.0)

            # ---- input loads: fine-grained multi-dim DMAs over two issue
            # queues, ordered by first use
            def dbox(t, s0, ns, p0, np_, c0, nc_):
                a = t.ap()
                S, Pp, C = a.ap[0][0], a.ap[1][0], a.ap[2][0]
                return bass.AP(
                    tensor=a.tensor,
                    offset=a.offset + s0 * S + p0 * Pp + c0 * C,
                    ap=[[Pp, np_], [S, ns], [C, nc_]])

            # m-tile column chunks for the k projection (also the DMA
            # split points for ctxT): first the two tiles the stream
            # opens with, then the rest
            c_t01 = min(2 * P, mpad)
            mchunks = [(0, c_t01)]
            off = c_t01
            while off < mpad:
                w = min(NBLK, mpad - off)
                mchunks.append((off, w))
                off += w

            # one queue, strict FIFO, ordered by first use; ~250KB+ per
            # DMA so the wire (not the ~625ns/DMA issue rate) is the
            # limiter.  The first query half comes in two kq-halves so
            # the first projection matmuls start while the wire runs.
            nc.sync.dma_start(out=xT_sb[:, 0:2, 0:NBLK],
                              in_=dbox(xT_d, 0, 2, 0, P, 0, NBLK))
            nc.sync.dma_start(out=wq_sb[:, :, 0:P],
                              in_=dbox(wq_d, 0, 4, 0, P, 0, P))
            nc.sync.dma_start(out=xT_sb[:, 2:4, 0:NBLK],
                              in_=dbox(xT_d, 2, 2, 0, P, 0, NBLK))
            nc.sync.dma_start(out=ctxT_sb[:, :, 0:c_t01],
                              in_=dbox(ctxT_d, 0, 4, 0, P, 0, c_t01))
            nc.sync.dma_start(out=wk_sb[:, :, 0:P],
                              in_=dbox(wk_d, 0, 4, 0, P, 0, P))
            nc.sync.dma_start(out=mb_sb[:], in_=mb_d[:])

            # remaining inputs, same queue (strict FIFO keeps the wire in
            # priority order), ordered by first use
            for coff, cw in mchunks[1:]:
                nc.sync.dma_start(out=ctxT_sb[:, :, coff:coff + cw],
                                  in_=dbox(ctxT_d, 0, 4, 0, P, coff, cw))
            nc.sync.dma_start(out=wv_sb[:],
                              in_=dbox(wv_d, 0, 4, 0, P, 0, INNER))
            nc.sync.dma_start(out=wq_sb[:, :, P:INNER],
                              in_=dbox(wq_d, 0, 4, 0, P, P, INNER - P))
            nc.sync.dma_start(out=wk_sb[:, :, P:INNER],
                              in_=dbox(wk_d, 0, 4, 0, P, P, INNER - P))
            nc.sync.dma_start(out=xT_sb[:, :, NBLK:NCORE],
                              in_=dbox(xT_d, 0, 4, 0, P, NBLK, NCORE - NBLK))
            nc.sync.dma_start(out=wo_sb[:],
                              in_=dbox(wo_d, 0, 4, 0, P, 0, QD))
            nc.sync.dma_start(out=wo3b[:], in_=dbox(wo_d, 3, 1, D, D, 0, QD))
            bo_src = bass.AP(tensor=bo_d.ap().tensor, offset=bo_d.ap().offset,
                             ap=[[0, P]] + bo_d.ap().ap[1:])
            nc.sync.dma_start(out=bo_bc[:], in_=bo_src)

            # ones columns for the softmax denominator (v copies leave them)
            nc.vector.memset(v8_sb[:, :, :, :, D:D + 1], 1.0)

            with tc.tile_pool(name="aux", bufs=2, space="PSUM") as aux, \
                 tc.tile_pool(name="sps", bufs=2, space="PSUM") as sps, \
                 tc.tile_pool(name="ops", bufs=1, space="PSUM") as ops, \
                 tc.tile_pool(name="ppool", bufs=5) as ppool, \
                 tc.tile_pool(name="raw", bufs=4) as rawp, \
                 tc.tile_pool(name="bcp", bufs=2) as bcp, \
                 tc.tile_pool(name="dscr", bufs=4, space="DRAM") as dscr:

                # a few throwaway matmuls ramp the PE p-state while the
                # first input tiles are still in flight, and a throwaway
                # exp pulls the activation table load off the critical path
                for _ in range(4):
                    pw = aux.tile([P, NBLK], f32, tag="aux", name="warm")
                    nc.tensor.matmul(pw[:], lhsT=wm_sb[:, 0:P],
                                     rhs=wm_sb[:], start=True, stop=True)
                nc.scalar.activation(out=dm_sb[0:1, :], in_=wm_sb[0:1, 0:8],
                                     func=EXP, scale=1.0)

                def v_unit(mt):
                    def f():
                        ps = aux.tile([P, INNER], f32, tag="aux", name="psv")
                        for kq in range(4):
                            nc.tensor.matmul(
                                ps[:],
                                lhsT=ctxT_sb[:, kq, mt * P:(mt + 1) * P],
                                rhs=wv_sb[:, kq, :],
                                start=(kq == 0), stop=(kq == 3),
                            )
                        psh = ps.rearrange("p (h d) -> p h d", h=H)
                        nc.vector.tensor_copy(
                            v8_sb[:, mt // 2, mt % 2, :, 0:D], psh[:])
                    return f

                def proj_unit_q(mi, nh):
                    def f():
                        ps = aux.tile([P, NBLK], f32, tag="aux", name="psq")
                        for kq in range(4):
                            nc.tensor.matmul(
                                ps[:],
                                lhsT=wq_sb[:, kq, mi * P:(mi + 1) * P],
                                rhs=xT_sb[:, kq, nh * NBLK:(nh + 1) * NBLK],
                                start=(kq == 0), stop=(kq == 3),
                            )
                        nc.vector.tensor_copy(
                            qT_sb[:, mi, nh * NBLK:(nh + 1) * NBLK], ps[:])
                    return f

                def proj_unit_k(mi, off, w):
                    def f():
                        ps = aux.tile([P, NBLK], f32, tag="aux", name="psk")
                        for kq in range(4):
                            nc.tensor.matmul(
                                ps[:, 0:w],
                                lhsT=wk_sb[:, kq, mi * P:(mi + 1) * P],
                                rhs=ctxT_sb[:, kq, off:off + w],
                                start=(kq == 0), stop=(kq == 3),
                            )
                        nc.vector.tensor_copy(
                            kT_sb[:, mi, off:off + w], ps[:, 0:w])
                    return f

                def fin_unit4(nt):
                    # out-proj for 128 queries: all four head-pairs chained
                    # into one PSUM accumulation, one bias-add, one DMA out
                    def f():
                        ps = aux.tile([P, NBLK], f32, tag="aux", name="psf")
                        for pp in range(4):
                            nc.tensor.matmul(
                                ps[:, 0:QD],
                                lhsT=o_sb[:, pp, nt * P:(nt + 1) * P],
                                rhs=wo_sb[:, pp, :],
                                start=(pp == 0), stop=(pp == 3),
                            )
                        nc.vector.tensor_add(
                            fin_sb[:, nt, :], ps[:, 0:QD], bo_bc[:])
                        nc.sync.dma_start(
                            out=out_d[nt * P:(nt + 1) * P, :],
                            in_=fin_sb[:, nt, :])
                    return f

                def fin_unit3(nt):
                    # out-proj partial for the last nb: pairs 0-2 chained,
                    # bias-added into fin_sb; pair 3 lands in the tail
                    def f():
                        ps = aux.tile([P, NBLK], f32, tag="aux", name="psf")
                        for pp in range(3):
                            nc.tensor.matmul(
                                ps[:, 0:QD],
                                lhsT=o_sb[:, pp, nt * P:(nt + 1) * P],
                                rhs=wo_sb[:, pp, :],
                                start=(pp == 0), stop=(pp == 2),
                            )
                        nc.vector.tensor_add(
                            fin_sb[:, nt, :], ps[:, 0:QD], bo_bc[:])
                    return f

                def fin_tail_pair(nt0, tb, c0):
                    # pair 3 of the final half for two 128-query blocks,
                    # straight from the normalize result via split-K (no
                    # shift DMA), matmuls hoisted ahead of the adds so
                    # add/DMA pipeline behind the PE
                    pss = []
                    for nt in (nt0, nt0 + 1):
                        tof = (nt - 4) * P - c0
                        ps = aux.tile([P, NBLK], f32, tag="aux", name="psf")
                        nc.tensor.matmul(
                            ps[:, 0:QD],
                            lhsT=o_sb[0:D, 3, nt * P:(nt + 1) * P],
                            rhs=wo_sb[0:D, 3, :],
                            start=True, stop=False,
                        )
                        nc.tensor.matmul(
                            ps[:, 0:QD],
                            lhsT=tb[0:D, tof:tof + P],
                            rhs=wo3b[:],
                            start=False, stop=True,
                        )
                        pss.append(ps)
                    for nt, ps in zip((nt0, nt0 + 1), pss):
                        nc.vector.tensor_add(
                            fin_sb[:, nt, :], ps[:, 0:QD], fin_sb[:, nt, :])
                        nc.sync.dma_start(
                            out=out_d[nt * P:(nt + 1) * P, :],
                            in_=fin_sb[:, nt, :])

                def fin_tail_a(nt):
                    # pair 3 of the first half: plain K=128 matmul off the
                    # shifted o_sb, dripped under the second half's stream
                    def f():
                        ps = aux.tile([P, NBLK], f32, tag="aux", name="psf")
                        nc.tensor.matmul(
                            ps[:, 0:QD],
                            lhsT=o_sb[:, 3, nt * P:(nt + 1) * P],
                            rhs=wo_sb[:, 3, :],
                            start=True, stop=True,
                        )
                        nc.vector.tensor_add(
                            fin_sb[:, nt, :], ps[:, 0:QD], fin_sb[:, nt, :])
                        nc.sync.dma_start(
                            out=out_d[nt * P:(nt + 1) * P, :],
                            in_=fin_sb[:, nt, :])
                    return f

                # Aux-work schedule. Every subblock has:
                #   prep[nb][p]: units emitted inline before its first
                #     score (data its own stream needs that could not be
                #     dripped earlier, e.g. DMA not yet landed)
                #   drip[nb][p]: units popped into the stream, at most
                #     `pops` per m-tile, emission order tracking expected
                #     DMA arrival so no waiting unit clogs the in-order
                #     PE queue ahead of ready work.  Anything left is
                #     flushed at the subblock end (before the drain pvs).
                def kprep(p, tail=False):
                    chs = mchunks[1:] if tail else mchunks
                    return [proj_unit_k(p, coff, cw) for coff, cw in chs]

                def stagger(start, units, step=1):
                    return [(start + i * step, u) for i, u in enumerate(units)]

                nv0 = min(7, nmt)   # v tiles dripped in p0; rest go to p1
                # pair 0's first chunk split per tile: score(t0) then only
                # waits on a 128-col k projection
                k0chunks = ([(0, P), (P, c_t01 - P)] if c_t01 > P
                            else [(0, c_t01)])
                prep = {(0, 0): [proj_unit_q(0, 0)]
                               + [proj_unit_k(0, co, cw_)
                                  for co, cw_ in k0chunks]}
                drip = {
                    # p0: its own late k chunks first (ctxT lands just in
                    # time), then the v tiles once wv has landed, then
                    # pair 1's projections (weights land last)
                    (0, 0): stagger(1, kprep(0, tail=True), 2)
                            + [(2 + mt // 2, v_unit(mt))
                               for mt in range(nv0)]
                            + [(6, proj_unit_q(1, 0)),
                               (7, proj_unit_k(1, *mchunks[0]))],
                    (0, 1): [(0, v_unit(mt)) for mt in range(nv0, nmt)]
                            + stagger(0, kprep(1, tail=True))
                            + stagger(3, [proj_unit_q(2, 0)] + kprep(2)),
                    (0, 2): stagger(0, [proj_unit_q(3, 0)] + kprep(3))
                            + [(4, proj_unit_q(0, 1))],
                    (0, 3): [],
                    (1, 0): [(0, proj_unit_q(1, 1))]
                            + [(nt + 3, fin_unit4(nt)) for nt in range(4)],
                    (1, 1): [(0, proj_unit_q(2, 1))],
                    (1, 2): [(0, proj_unit_q(3, 1))],
                    (1, 3, "b"): [(nt - 2, fin_unit3(nt))
                                  for nt in range(4, 8)],
                }

                # each subblock defers its last pv pairs + normalize into
                # the next subblock's drip slots (the next stream's early
                # m-tiles are PE-light), so pair boundaries don't stall
                # the exp stream on a serial drain
                # subblock sequence; the last subblock takes the "b"
                # latency-optimized drain
                seq = ([(0, pq, 0, NBLK, "n") for pq in range(4)]
                       + [(1, pq, 0, NBLK, "n") for pq in range(3)]
                       + [(1, 3, 0, NBLK, "b")])

                carry = []
                final = []
                for si, (nb, p, c0, cw, kind) in enumerate(seq):
                    if si == 0:
                        for u in prep.get((0, 0), []):
                            u()
                    pending = sorted(drip.get((nb, p, kind),
                                              drip.get((nb, p), []))
                                     + carry, key=lambda e: e[0])
                    carry = []

                    nsl = slice(nb * NBLK + c0, nb * NBLK + c0 + cw)
                    oa = ops.tile([P, NBLK], f32, tag="oa")
                    ob = ops.tile([P, NBLK], f32, tag="ob")
                    pts = {}
                    npv = [0]

                    def pv(tp, p=p, oa=oa, ob=ob, pts=pts, npv=npv, cw=cw):
                        # one DoubleRow matmul per head covers both
                        # m-tiles of the pair (K=256); an odd trailing
                        # tile (tp == ntp) uses a normal fp8 matmul
                        pt = pts.pop(tp)
                        first = npv[0] == 0
                        last = tp == (ntp + leftover - 1)
                        npv[0] += 1
                        if tp < ntp:
                            for h, ot in ((0, oa), (1, ob)):
                                nc.tensor.matmul(
                                    ot[0:D + 1, 0:cw],
                                    lhsT=v8_sb[:, tp, :, 2 * p + h, 0:D + 1],
                                    rhs=pt[:, :, h, 0:cw],
                                    start=first, stop=last,
                                    perf_mode=DR,
                                )
                        else:
                            for h, ot in ((0, oa), (1, ob)):
                                nc.tensor.matmul(
                                    ot[0:D + 1, 0:cw],
                                    lhsT=v8_sb[:, tp, 0, 2 * p + h, 0:D + 1],
                                    rhs=pt[:, 0, h, 0:cw],
                                    start=first, stop=last,
                                )

                    for mt in range(nmt):
                        tp, ti = mt // 2, mt % 2
                        if ti == 0:
                            pts[tp] = ppool.tile([P, 2, 2, NBLK], fp8,
                                                 tag="pt", name="pt")
                        sp = sps.tile([P, 2 * NBLK], f32, tag="s")
                        msl = slice(mt * P, (mt + 1) * P)
                        nc.tensor.matmul(
                            sp[:, 0:cw],
                            lhsT=kT_sb[0:64, p, msl],
                            rhs=qT_sb[0:64, p, nsl],
                            start=True, stop=True,
                        )
                        nc.tensor.matmul(
                            sp[:, cw:2 * cw],
                            lhsT=kT_sb[64:128, p, msl],
                            rhs=qT_sb[64:128, p, nsl],
                            start=True, stop=True,
                        )
                        # exp writes P^T as fp8 [head, n] for this tile
                        sph = sp[:, 0:2 * cw].rearrange(
                            "q (h n) -> q h n", h=2)
                        nc.scalar.activation(
                            out=pts[tp][:, ti, :, 0:cw], in_=sph[:],
                            func=EXP,
                            bias=mb_sb[:, mt:mt + 1], scale=SCALE,
                        )
                        # PV lags the exp stream so the in-order PE
                        # queue never stalls on the ACT stream (deeper
                        # lag in the very first subblock, where the v
                        # tiles are still being produced)
                        lag = 4 if si == 0 else 3
                        if mt >= lag and (mt - lag) % 2 == 1:
                            pv((mt - lag) // 2)
                        pops = 0
                        while (pending and pending[0][0] <= mt
                               and pops < 2):
                            pending.pop(0)[1]()
                            pops += 1
                    for _, u in pending:
                        u()

                    def normalize(p=p, nsl=nsl, oa=oa, ob=ob, cw=cw):
                        # normalize: bounce the raw denominator rows
                        # through DRAM for the partition broadcast,
                        # reciprocal out of place, then multiply
                        rawa = rawp.tile([P, NBLK], f32, tag="rawa",
                                         name="rawa")
                        rawb = rawp.tile([P, NBLK], f32, tag="rawb",
                                         name="rawb")
                        tb = rawp.tile([D, NBLK], bf16, tag="tb",
                                       name="tb")
                        bcb = bcp.tile([D, 2, NBLK], f32, tag="bcb",
                                       name="bcb")
                        rcb = bcp.tile([D, 2, NBLK], f32, tag="rcb",
                                       name="rcb")
                        scr = dscr.tile([2, NBLK], f32, tag="scr",
                                        name="scr")
                        nc.vector.tensor_copy(rawa[0:D + 1, 0:cw],
                                              oa[0:D + 1, 0:cw])
                        nc.vector.tensor_copy(rawb[0:D + 1, 0:cw],
                                              ob[0:D + 1, 0:cw])
                        for i, raw in ((0, rawa), (1, rawb)):
                            nc.sync.dma_start(out=scr[i:i + 1, 0:cw],
                                              in_=raw[D:D + 1, 0:cw])
                            src = scr[i:i + 1, 0:cw]
                            bsrc = bass.AP(tensor=src.tensor,
                                           offset=src.offset,
                                           ap=[[0, D]] + src.ap[1:])
                            nc.sync.dma_start(out=rcb[0:D, i, 0:cw],
                                              in_=bsrc)
                        nc.vector.reciprocal_approx_fast(
                            out=bcb[0:D, :, 0:cw], in_=rcb[0:D, :, 0:cw])
                        nc.vector.tensor_mul(
                            o_sb[0:D, p, nsl], rawa[0:D, 0:cw],
                            bcb[0:D, 0, 0:cw])
                        nc.vector.tensor_mul(
                            tb[0:D, 0:cw], rawb[0:D, 0:cw],
                            bcb[0:D, 1, 0:cw])
                        nc.sync.dma_start(out=o_sb[D:P, p, nsl],
                                          in_=tb[0:D, 0:cw])

                    if kind != "b":
                        # the deferred pvs and normalize must all pop
                        # before the next subblock's own first pv
                        # (which reuses the single-buffered oa/ob)
                        rem = sorted(pts)
                        carry = [(i // 2, (lambda tp=tp, pv=pv: pv(tp)))
                                 for i, tp in enumerate(rem)]
                        carry.append(((len(rem) + 1) // 2, normalize))
                    else:
                        # final half: latency-optimized drain. ACT (its
                        # exp stream just ended) reads only the
                        # denominator rows, the idle PE broadcasts them,
                        # the muls read the PV accumulators straight from
                        # PSUM, and pair 3 of the out-proj takes the
                        # second head half straight from tb
                        for tp in sorted(pts):
                            pv(tp)
                        tb = rawp.tile([D, NBLK], bf16, tag="tb",
                                       name="tb")
                        bcb = bcp.tile([D, 2, NBLK], f32, tag="bcb",
                                       name="bcb")
                        den = bcp.tile([P, 2, NBLK], bf16, tag="den")
                        nc.scalar.activation(out=den[D:D + 1, 0, 0:cw],
                                             in_=oa[D:D + 1, 0:cw], func=CPY)
                        nc.scalar.activation(out=den[D:D + 1, 1, 0:cw],
                                             in_=ob[D:D + 1, 0:cw], func=CPY)
                        bca = aux.tile([P, NBLK], f32, tag="aux",
                                       name="bca")
                        bcq = aux.tile([P, NBLK], f32, tag="aux",
                                       name="bcq")
                        nc.tensor.matmul(
                            bca[0:D, 0:cw], lhsT=ones_sb[D:D + 1, :],
                            rhs=den[D:D + 1, 0, 0:cw],
                            start=True, stop=True)
                        nc.tensor.matmul(
                            bcq[0:D, 0:cw], lhsT=ones_sb[D:D + 1, :],
                            rhs=den[D:D + 1, 1, 0:cw],
                            start=True, stop=True)
                        HB = cw // 2
                        nc.vector.reciprocal_approx_fast(
                            out=bcb[0:D, 0, 0:cw], in_=bca[0:D, 0:cw])
                        for hh in range(2):
                            hsl = slice(hh * HB, (hh + 1) * HB)
                            osl = slice(nb * NBLK + c0 + hh * HB,
                                        nb * NBLK + c0 + (hh + 1) * HB)
                            nc.vector.tensor_mul(
                                o_sb[0:D, 3, osl], oa[0:D, hsl],
                                bcb[0:D, 0, hsl])
                            if hh == 0:
                                nc.vector.reciprocal_approx_fast(
                                    out=bcb[0:D, 1, 0:cw],
                                    in_=bcq[0:D, 0:cw])
                            nc.vector.tensor_mul(
                                tb[0:D, hsl], ob[0:D, hsl],
                                bcb[0:D, 1, hsl])
                            final.append(
                                lambda nt0=4 + (c0 + hh * HB) // P, tb=tb:
                                fin_tail_pair(nt0, tb, c0))
                # drain: the final half's out-proj tail
                for f in final:
                    f()

    nc.compile()
    return nc


def get_nc(nmt=None):
    if nmt is None:
        nmt = _CACHE.get("last_nmt", M // P)
    if ("nc", nmt) not in _CACHE:
        _CACHE[("nc", nmt)] = _build_nc(nmt)
    _CACHE["last_nmt"] = nmt
    return _CACHE[("nc", nmt)]


def make_in_maps(x, context, mask, Wq, Wkv, Wo, bo):
    """CPU glue: shard, transpose, cast, and compact keys by mask."""
    bf = ml_dtypes.bfloat16
    Wk = np.ascontiguousarray(Wkv[:, :INNER]).astype(bf)
    Wv = np.ascontiguousarray(Wkv[:, INNER:]).astype(bf)
    Wq_b = np.ascontiguousarray(Wq).astype(bf)
    Wo_b = np.ascontiguousarray(Wo).astype(bf)
    bo_f = np.ascontiguousarray(bo, dtype=np.float32).reshape(1, QD)

    idxs = [np.where(mask[b])[0] for b in range(B)]
    maxc = max(1, max(len(i) for i in idxs))
    nmt = (maxc + P - 1) // P
    mpad = nmt * P

    in_maps = []
    for c in range(8):
        b, s = c // 2, c % 2
        idx = idxs[b]
        cnt = len(idx)
        ctx_c = np.zeros((mpad, QD), dtype=np.float32)
        ctx_c[:cnt] = context[b][idx]
        mb = np.full(mpad, MASK_NEG, dtype=np.float32)
        mb[:cnt] = 0.0
        xT = np.ascontiguousarray(
            x[b, s * NCORE:(s + 1) * NCORE, :].T).astype(bf)
        ctxT = np.ascontiguousarray(ctx_c.T).astype(bf)
        mbt = np.ascontiguousarray(mb.reshape(nmt, P).T)
        in_maps.append({
            "xT": xT.reshape(4, P, NCORE),
            "ctxT": ctxT.reshape(4, P, mpad),
            "wq": Wq_b.reshape(4, P, INNER),
            "wk": Wk.reshape(4, P, INNER),
            "wv": Wv.reshape(4, P, INNER),
            "wo": Wo_b.reshape(4, P, QD),
            "bo": bo_f, "mb": mbt,
        })
    return in_maps, nmt


def assemble(results):
    out = np.empty((B, N, QD), dtype=np.float32)
    for c in range(8):
        b, s = c // 2, c % 2
        out[b, s * NCORE:(s + 1) * NCORE, :] = np.asarray(
            results[c]["out"], dtype=np.float32)
    return out


def kernel(x, context, mask, Wq, Wkv, Wo, bo):
    from concourse.bass_utils import run_bass_kernel_spmd

    x = np.asarray(x, dtype=np.float32)
    context = np.asarray(context, dtype=np.float32)
    mask = np.asarray(mask)
    in_maps, nmt = make_in_maps(x, context, mask,
                                np.asarray(Wq, dtype=np.float32),
                                np.asarray(Wkv, dtype=np.float32),
                                np.asarray(Wo, dtype=np.float32),
                                np.asarray(bo, dtype=np.float32))
    nc = get_nc(nmt)
    res = run_bass_kernel_spmd(nc, in_maps, list(range(8)))
    return assemble(res.results)


# revision 61
# speedup vs baseline: 1.0058x; 1.0035x over previous
"""Trainium2 Bass kernel for nn_Attention_3375844294750.

Cross-attention (q from x, k/v from context) with key mask, 8 heads, d=64.
  B=4, N=M=2048, query_dim=context_dim=512, inner=512.

Sharding: 8 NeuronCores = (batch b = core//2) x (query-half = core%2).
Each core computes attention for its 1024 queries over its batch's keys.
No collectives needed (outputs are disjoint).

Key compaction: masked keys contribute exactly 0 to masked softmax, so the
CPU glue gathers only the unmasked keys (~50% of 2048) per batch, padded
to a multiple of 128; padding slots are killed by the exp bias.

Structure (nb outer, head-pair inner):
  for nb in (0, 1):            # query half of this core's 1024 queries
    for p in (0..3):           # head pair
      stream over m-tiles: S^T -> exp (fp8 out) -> PV
PV uses fp8e4 DoubleRow matmuls over m-tile PAIRS (K=256 per
instruction, ~1.5-2x the bf16 matmul rate): the ACT exp writes P^T
directly as fp8e4 into a [tile,head,n] layout whose pair stride is
16B-aligned, and V is stored fp8 with a ones column so the softmax
denominator rides along as PSUM row 64.  The unnormalized exp values
(logits bounded, no max subtraction) sit in e4m3's sweet range, and the
denominator is computed from the same quantized P, cancelling the
common-mode quantization error.  Scores stay bf16 (a K=64 contraction
gets no DoubleRow win on hardware).  The out-projection accumulates all
four head-pairs in one PSUM tile via a chained matmul group (one DVE
bias-add per 128-query block instead of one add per pair); nb0's
out-proj + output DMA drip into nb1's PE slack.

Scheduling: each subblock's projection/out-proj prep is dripped into
earlier subblocks' PE slack (at most two units per m-tile, emission
order tracking expected DMA arrival so waiting matmuls never clog the
in-order PE queue), and each subblock's last PV pairs + normalize are
deferred into the NEXT subblock's early, PE-light m-tiles so pair
boundaries never stall the exp stream on a serial drain.  The
normalize broadcasts denominator rows across partitions via a DRAM
bounce hidden under the next subblock (gpsimd partition_broadcast
passes CoreSim but returns garbage on HW).  The last subblock keeps a
latency-optimized drain: ACT copies only the denominator rows, the
then-idle PE broadcasts them, the muls read the PV accumulators
straight from PSUM, and pair 3 of the out-projection takes the second
head half straight from the normalize result via a split-K
accumulation.  Output is stored bf16 to halve the final DMA.

Startup: inputs load on one strict-FIFO DGE queue ordered by first use
at ~250KB+ granularity (so the wire, not the ~650ns/DMA issue rate, is
the limiter), the first query half split in two kq-halves and pair 0's
first k-chunk split per tile so the first score waits on as little as
possible; throwaway matmuls ramp the PE clock and a throwaway exp pulls
the activation-table load off the critical path.
"""
import os
import sys

for _p in ("/opt/trn_rl_repo", "/root/.axon_site/_ro/trn_rl_repo"):
    if os.path.isdir(_p) and _p not in sys.path:
        sys.path.insert(0, _p)
        break

import numpy as np
import ml_dtypes

B, N, M = 4, 2048, 2048
QD = 512          # query_dim == context_dim
H, D = 8, 64
INNER = H * D     # 512
SCALE = D ** -0.5
NCORE = N // 2    # queries per core = 1024
P = 128
NBLK = 512        # n-block (one PSUM bank per matmul)
MASK_NEG = -1e30

_CACHE = {}


def _build_nc(nmt):
    """Build + compile the SPMD program for nmt m-tiles (m_pad = 128*nmt)."""
    import concourse.mybir as mybir
    from concourse import bacc
    from concourse.tile import TileContext
    import concourse.bass as bass

    mpad = nmt * P
    ntp = nmt // 2           # full m-tile pairs for DoubleRow PV
    leftover = nmt % 2       # odd trailing m-tile
    dt = mybir.dt
    nc = bacc.Bacc("TRN2", target_bir_lowering=False, debug=False, num_devices=8)

    xT_d = nc.declare_dram_parameter("xT", [4, P, NCORE], dt.bfloat16, isOutput=False)
    ctxT_d = nc.declare_dram_parameter("ctxT", [4, P, mpad], dt.bfloat16, isOutput=False)
    wq_d = nc.declare_dram_parameter("wq", [4, P, INNER], dt.bfloat16, isOutput=False)
    wk_d = nc.declare_dram_parameter("wk", [4, P, INNER], dt.bfloat16, isOutput=False)
    wv_d = nc.declare_dram_parameter("wv", [4, P, INNER], dt.bfloat16, isOutput=False)
    wo_d = nc.declare_dram_parameter("wo", [4, P, QD], dt.bfloat16, isOutput=False)
    bo_d = nc.declare_dram_parameter("bo", [1, QD], dt.float32, isOutput=False)
    mb_d = nc.declare_dram_parameter("mb", [P, nmt], dt.float32, isOutput=False)
    out_d = nc.declare_dram_parameter("out", [NCORE, QD], dt.bfloat16, isOutput=True)

    f32 = dt.float32
    bf16 = dt.bfloat16
    fp8 = dt.float8e4
    EXP = mybir.ActivationFunctionType.Exp
    CPY = mybir.ActivationFunctionType.Copy
    DR = mybir.MatmulPerfMode.DoubleRow

    # v tile free-dim stride: D+1 (values + ones row), padded so the
    # DoubleRow pair stride (H*VS bytes, fp8) is a multiple of 16
    VS = 80

    with TileContext(nc) as tc:
        from contextlib import ExitStack

        with ExitStack() as ctx:
            const = ctx.enter_context(tc.tile_pool(name="const", bufs=1))

            # ---- persistent SBUF tensors ----
            wq_sb = const.tile([P, 4, INNER], bf16, tag="wq")
            xT_sb = const.tile([P, 4, NCORE], bf16, tag="xT")
            wk_sb = const.tile([P, 4, INNER], bf16, tag="wk")
            ctxT_sb = const.tile([P, 4, mpad], bf16, tag="cT")
            wv_sb = const.tile([P, 4, INNER], bf16, tag="wv")
            wo_sb = const.tile([P, 4, QD], bf16, tag="wo")
            bo_bc = const.tile([P, QD], f32, tag="bo")
            mb_sb = const.tile([P, nmt], f32, tag="mb")
            wm_sb = const.tile([P, NBLK], bf16, tag="wm")

            ones_sb = const.tile([P, D], bf16, tag="ones")
            dm_sb = const.tile([P, 8], bf16, tag="dm")
            wo3b = const.tile([D, QD], bf16, tag="wo3b")
            qT_sb = const.tile([P, 4, NCORE], bf16, tag="qT")
            kT_sb = const.tile([P, 4, mpad], bf16, tag="kT")
            # fp8 V with ones column, [tilepair][tile-in-pair][head][VS]
            nvt = ntp + leftover
            v8_sb = const.tile([P, nvt, 2, H, VS], fp8, tag="v8")
            o_sb = const.tile([P, 4, NCORE], bf16, tag="oT")
            fin_sb = const.tile([P, NCORE // P, QD], bf16, tag="fin")

            # warm-up weights for the PE clock ramp (never read by output)
            nc.vector.memset(wm_sb[:], 1.0)
            nc.vector.memset(ones_sb[:], 1.0)

            # ---- input loads: fine-grained multi-dim DMAs over two issue
            # queues, ordered by first use
            def dbox(t, s0, ns, p0, np_, c0, nc_):
                a = t.ap()
                S, Pp, C = a.ap[0][0], a.ap[1][0], a.ap[2][0]
                return bass.AP(
                    tensor=a.tensor,
                    offset=a.offset + s0 * S + p0 * Pp + c0 * C,
                    ap=[[Pp, np_], [S, ns], [C, nc_]])

            # m-tile column chunks for the k projection (also the DMA
            # split points for ctxT): first the two tiles the stream
            # opens with, then the rest
            c_t01 = min(2 * P, mpad)
            mchunks = [(0, c_t01)]
            off = c_t01
            while off < mpad:
                w = min(NBLK, mpad - off)
                mchunks.append((off, w))
                off += w

            # one queue, strict FIFO, ordered by first use; ~250KB+ per
            # DMA so the wire (not the ~625ns/DMA issue rate) is the
            # limiter.  The first query half comes in two kq-halves so
            # the first projection matmuls start while the wire runs.
            nc.sync.dma_start(out=xT_sb[:, 0:2, 0:NBLK],
                              in_=dbox(xT_d, 0, 2, 0, P, 0, NBLK))
            nc.sync.dma_start(out=wq_sb[:, :, 0:P],
                              in_=dbox(wq_d, 0, 4, 0, P, 0, P))
            nc.sync.dma_start(out=xT_sb[:, 2:4, 0:NBLK],
                              in_=dbox(xT_d, 2, 2, 0, P, 0, NBLK))
            nc.sync.dma_start(out=ctxT_sb[:, :, 0:c_t01],
                              in_=dbox(ctxT_d, 0, 4, 0, P, 0, c_t01))
            nc.sync.dma_start(out=wk_sb[:, :, 0:P],
                              in_=dbox(wk_d, 0, 4, 0, P, 0, P))
            nc.sync.dma_start(out=mb_sb[:], in_=mb_d[:])

            # remaining inputs, same queue (strict FIFO keeps the wire in
            # priority order), ordered by first use
            for coff, cw in mchunks[1:]:
                nc.sync.dma_start(out=ctxT_sb[:, :, coff:coff + cw],
                                  in_=dbox(ctxT_d, 0, 4, 0, P, coff, cw))
            nc.sync.dma_start(out=wv_sb[:],
                              in_=dbox(wv_d, 0, 4, 0, P, 0, INNER))
            nc.sync.dma_start(out=wq_sb[:, :, P:INNER],
                              in_=dbox(wq_d, 0, 4, 0, P, P, INNER - P))
            nc.sync.dma_start(out=wk_sb[:, :, P:INNER],
                              in_=dbox(wk_d, 0, 4, 0, P, P, INNER - P))
            nc.sync.dma_start(out=xT_sb[:, :, NBLK:NCORE],
                              in_=dbox(xT_d, 0, 4, 0, P, NBLK, NCORE - NBLK))
            nc.sync.dma_start(out=wo_sb[:],
                              in_=dbox(wo_d, 0, 4, 0, P, 0, QD))
            nc.sync.dma_start(out=wo3b[:], in_=dbox(wo_d, 3, 1, D, D, 0, QD))
            bo_src = bass.AP(tensor=bo_d.ap().tensor, offset=bo_d.ap().offset,
                             ap=[[0, P]] + bo_d.ap().ap[1:])
            nc.sync.dma_start(out=bo_bc[:], in_=bo_src)

            # ones columns for the softmax denominator (v copies leave them)
            nc.vector.memset(v8_sb[:, :, :, :, D:D + 1], 1.0)

            with tc.tile_pool(name="aux", bufs=2, space="PSUM") as aux, \
                 tc.tile_pool(name="sps", bufs=2, space="PSUM") as sps, \
                 tc.tile_pool(name="ops", bufs=1, space="PSUM") as ops, \
                 tc.tile_pool(name="ppool", bufs=5) as ppool, \
                 tc.tile_pool(name="raw", bufs=4) as rawp, \
                 tc.tile_pool(name="bcp", bufs=2) as bcp, \
                 tc.tile_pool(name="dscr", bufs=4, space="DRAM") as dscr:

                # a few throwaway matmuls ramp the PE p-state while the
                # first input tiles are still in flight, and a throwaway
                # exp pulls the activation table load off the critical path
                for _ in range(4):
                    pw = aux.tile([P, NBLK], f32, tag="aux", name="warm")
                    nc.tensor.matmul(pw[:], lhsT=wm_sb[:, 0:P],
                                     rhs=wm_sb[:], start=True, stop=True)
                nc.scalar.activation(out=dm_sb[0:1, :], in_=wm_sb[0:1, 0:8],
                                     func=EXP, scale=1.0)

                def v_unit(mt):
                    def f():
                        ps = aux.tile([P, INNER], f32, tag="aux", name="psv")
                        for kq in range(4):
                            nc.tensor.matmul(
                                ps[:],
                                lhsT=ctxT_sb[:, kq, mt * P:(mt + 1) * P],
                                rhs=wv_sb[:, kq, :],
                                start=(kq == 0), stop=(kq == 3),
                            )
                        psh = ps.rearrange("p (h d) -> p h d", h=H)
                        nc.vector.tensor_copy(
                            v8_sb[:, mt // 2, mt % 2, :, 0:D], psh[:])
                    return f

                def proj_unit_q(mi, nh):
                    def f():
                        ps = aux.tile([P, NBLK], f32, tag="aux", name="psq")
                        for kq in range(4):
                            nc.tensor.matmul(
                                ps[:],
                                lhsT=wq_sb[:, kq, mi * P:(mi + 1) * P],
                                rhs=xT_sb[:, kq, nh * NBLK:(nh + 1) * NBLK],
                                start=(kq == 0), stop=(kq == 3),
                            )
                        nc.vector.tensor_copy(
                            qT_sb[:, mi, nh * NBLK:(nh + 1) * NBLK], ps[:])
                    return f

                def proj_unit_k(mi, off, w):
                    def f():
                        ps = aux.tile([P, NBLK], f32, tag="aux", name="psk")
                        for kq in range(4):
                            nc.tensor.matmul(
                                ps[:, 0:w],
                                lhsT=wk_sb[:, kq, mi * P:(mi + 1) * P],
                                rhs=ctxT_sb[:, kq, off:off + w],
                                start=(kq == 0), stop=(kq == 3),
                            )
                        nc.vector.tensor_copy(
                            kT_sb[:, mi, off:off + w], ps[:, 0:w])
                    return f

                def fin_unit4(nt):
                    # out-proj for 128 queries: all four head-pairs chained
                    # into one PSUM accumulation, one bias-add, one DMA out
                    def f():
                        ps = aux.tile([P, NBLK], f32, tag="aux", name="psf")
                        for pp in range(4):
                            nc.tensor.matmul(
                                ps[:, 0:QD],
                                lhsT=o_sb[:, pp, nt * P:(nt + 1) * P],
                                rhs=wo_sb[:, pp, :],
                                start=(pp == 0), stop=(pp == 3),
                            )
                        nc.vector.tensor_add(
                            fin_sb[:, nt, :], ps[:, 0:QD], bo_bc[:])
                        nc.sync.dma_start(
                            out=out_d[nt * P:(nt + 1) * P, :],
                            in_=fin_sb[:, nt, :])
                    return f

                def fin_unit3(nt):
                    # out-proj partial for the last nb: pairs 0-2 chained,
                    # bias-added into fin_sb; pair 3 lands in the tail
                    def f():
                        ps = aux.tile([P, NBLK], f32, tag="aux", name="psf")
                        for pp in range(3):
                            nc.tensor.matmul(
                                ps[:, 0:QD],
                                lhsT=o_sb[:, pp, nt * P:(nt + 1) * P],
                                rhs=wo_sb[:, pp, :],
                                start=(pp == 0), stop=(pp == 2),
                            )
                        nc.vector.tensor_add(
                            fin_sb[:, nt, :], ps[:, 0:QD], bo_bc[:])
                    return f

                def fin_tail_pair(nt0, tb, c0):
                    # pair 3 of the final half for two 128-query blocks,
                    # straight from the normalize result via split-K (no
                    # shift DMA), matmuls hoisted ahead of the adds so
                    # add/DMA pipeline behind the PE
                    pss = []
                    for nt in (nt0, nt0 + 1):
                        tof = (nt - 4) * P - c0
                        ps = aux.tile([P, NBLK], f32, tag="aux", name="psf")
                        nc.tensor.matmul(
                            ps[:, 0:QD],
                            lhsT=o_sb[0:D, 3, nt * P:(nt + 1) * P],
                            rhs=wo_sb[0:D, 3, :],
                            start=True, stop=False,
                        )
                        nc.tensor.matmul(
                            ps[:, 0:QD],
                            lhsT=tb[0:D, tof:tof + P],
                            rhs=wo3b[:],
                            start=False, stop=True,
                        )
                        pss.append(ps)
                    for nt, ps in zip((nt0, nt0 + 1), pss):
                        nc.vector.tensor_add(
                            fin_sb[:, nt, :], ps[:, 0:QD], fin_sb[:, nt, :])
                        nc.sync.dma_start(
                            out=out_d[nt * P:(nt + 1) * P, :],
                            in_=fin_sb[:, nt, :])

                def fin_tail_a(nt):
                    # pair 3 of the first half: plain K=128 matmul off the
                    # shifted o_sb, dripped under the second half's stream
                    def f():
                        ps = aux.tile([P, NBLK], f32, tag="aux", name="psf")
                        nc.tensor.matmul(
                            ps[:, 0:QD],
                            lhsT=o_sb[:, 3, nt * P:(nt + 1) * P],
                            rhs=wo_sb[:, 3, :],
                            start=True, stop=True,
                        )
                        nc.vector.tensor_add(
                            fin_sb[:, nt, :], ps[:, 0:QD], fin_sb[:, nt, :])
                        nc.sync.dma_start(
                            out=out_d[nt * P:(nt + 1) * P, :],
                            in_=fin_sb[:, nt, :])
                    return f

                # Aux-work schedule. Every subblock has:
                #   prep[nb][p]: units emitted inline before its first
                #     score (data its own stream needs that could not be
                #     dripped earlier, e.g. DMA not yet landed)
                #   drip[nb][p]: units popped into the stream, at most
                #     `pops` per m-tile, emission order tracking expected
                #     DMA arrival so no waiting unit clogs the in-order
                #     PE queue ahead of ready work.  Anything left is
                #     flushed at the subblock end (before the drain pvs).
                def kprep(p, tail=False):
                    chs = mchunks[1:] if tail else mchunks
                    return [proj_unit_k(p, coff, cw) for coff, cw in chs]

                def stagger(start, units, step=1):
                    return [(start + i * step, u) for i, u in enumerate(units)]

                nv0 = min(7, nmt)   # v tiles dripped in p0; rest go to p1
                # pair 0's first chunk split per tile: score(t0) then only
                # waits on a 128-col k projection
                k0chunks = ([(0, P), (P, c_t01 - P)] if c_t01 > P
                            else [(0, c_t01)])
                prep = {(0, 0): [proj_unit_q(0, 0)]
                               + [proj_unit_k(0, co, cw_)
                                  for co, cw_ in k0chunks]}
                drip = {
                    # p0: its own late k chunks first (ctxT lands just in
                    # time), then the v tiles once wv has landed, then
                    # pair 1's projections (weights land last)
                    (0, 0): stagger(1, kprep(0, tail=True), 2)
                            + [(2 + mt // 2, v_unit(mt))
                               for mt in range(nv0)]
                            + [(6, proj_unit_q(1, 0)),
                               (7, proj_unit_k(1, *mchunks[0]))],
                    (0, 1): [(0, v_unit(mt)) for mt in range(nv0, nmt)]
                            + stagger(0, kprep(1, tail=True))
                            + stagger(3, [proj_unit_q(2, 0),
                                          proj_unit_k(2, *mchunks[0])]),
                    (0, 2): stagger(0, kprep(2, tail=True))
                            + stagger(2, [proj_unit_q(3, 0),
                                          proj_unit_k(3, *mchunks[0])])
                            + [(4, proj_unit_q(0, 1))],
                    (0, 3): stagger(0, kprep(3, tail=True)),
                    (1, 0): [(0, proj_unit_q(1, 1))]
                            + [(nt + 3, fin_unit4(nt)) for nt in range(4)],
                    (1, 1): [(0, proj_unit_q(2, 1))],
                    (1, 2): [(0, proj_unit_q(3, 1))],
                    (1, 3, "b"): [(nt - 2, fin_unit3(nt))
                                  for nt in range(4, 8)],
                }

                # each subblock defers its last pv pairs + normalize into
                # the next subblock's drip slots (the next stream's early
                # m-tiles are PE-light), so pair boundaries don't stall
                # the exp stream on a serial drain
                # subblock sequence; the last subblock takes the "b"
                # latency-optimized drain
                seq = ([(0, pq, 0, NBLK, "n") for pq in range(4)]
                       + [(1, pq, 0, NBLK, "n") for pq in range(3)]
                       + [(1, 3, 0, NBLK, "b")])

                carry = []
                final = []
                for si, (nb, p, c0, cw, kind) in enumerate(seq):
                    if si == 0:
                        for u in prep.get((0, 0), []):
                            u()
                    pending = sorted(drip.get((nb, p, kind),
                                              drip.get((nb, p), []))
                                     + carry, key=lambda e: e[0])
                    carry = []

                    nsl = slice(nb * NBLK + c0, nb * NBLK + c0 + cw)
                    oa = ops.tile([P, NBLK], f32, tag="oa")
                    ob = ops.tile([P, NBLK], f32, tag="ob")
                    pts = {}
                    npv = [0]

                    def pv(tp, p=p, oa=oa, ob=ob, pts=pts, npv=npv, cw=cw):
                        # one DoubleRow matmul per head covers both
                        # m-tiles of the pair (K=256); an odd trailing
                        # tile (tp == ntp) uses a normal fp8 matmul
                        pt = pts.pop(tp)
                        first = npv[0] == 0
                        last = tp == (ntp + leftover - 1)
                        npv[0] += 1
                        if tp < ntp:
                            for h, ot in ((0, oa), (1, ob)):
                                nc.tensor.matmul(
                                    ot[0:D + 1, 0:cw],
                                    lhsT=v8_sb[:, tp, :, 2 * p + h, 0:D + 1],
                                    rhs=pt[:, :, h, 0:cw],
                                    start=first, stop=last,
                                    perf_mode=DR,
                                )
                        else:
                            for h, ot in ((0, oa), (1, ob)):
                                nc.tensor.matmul(
                                    ot[0:D + 1, 0:cw],
                                    lhsT=v8_sb[:, tp, 0, 2 * p + h, 0:D + 1],
                                    rhs=pt[:, 0, h, 0:cw],
                                    start=first, stop=last,
                                )

                    for mt in range(nmt):
                        tp, ti = mt // 2, mt % 2
                        if ti == 0:
                            pts[tp] = ppool.tile([P, 2, 2, NBLK], fp8,
                                                 tag="pt", name="pt")
                        sp = sps.tile([P, 2 * NBLK], f32, tag="s")
                        msl = slice(mt * P, (mt + 1) * P)
                        nc.tensor.matmul(
                            sp[:, 0:cw],
                            lhsT=kT_sb[0:64, p, msl],
                            rhs=qT_sb[0:64, p, nsl],
                            start=True, stop=True,
                        )
                        nc.tensor.matmul(
                            sp[:, cw:2 * cw],
                            lhsT=kT_sb[64:128, p, msl],
                            rhs=qT_sb[64:128, p, nsl],
                            start=True, stop=True,
                        )
                        # exp writes P^T as fp8 [head, n] for this tile
                        sph = sp[:, 0:2 * cw].rearrange(
                            "q (h n) -> q h n", h=2)
                        nc.scalar.activation(
                            out=pts[tp][:, ti, :, 0:cw], in_=sph[:],
                            func=EXP,
                            bias=mb_sb[:, mt:mt + 1], scale=SCALE,
                        )
                        # PV lags the exp stream so the in-order PE
                        # queue never stalls on the ACT stream (deeper
                        # lag in the very first subblock, where the v
                        # tiles are still being produced)
                        lag = 4 if si == 0 else 3
                        if mt >= lag and (mt - lag) % 2 == 1:
                            pv((mt - lag) // 2)
                        pops = 0
                        while (pending and pending[0][0] <= mt
                               and pops < 2):
                            pending.pop(0)[1]()
                            pops += 1
                    for _, u in pending:
                        u()

                    def normalize(p=p, nsl=nsl, oa=oa, ob=ob, cw=cw):
                        # normalize: bounce the raw denominator rows
                        # through DRAM for the partition broadcast,
                        # reciprocal out of place, then multiply
                        rawa = rawp.tile([P, NBLK], f32, tag="rawa",
                                         name="rawa")
                        rawb = rawp.tile([P, NBLK], f32, tag="rawb",
                                         name="rawb")
                        tb = rawp.tile([D, NBLK], bf16, tag="tb",
                                       name="tb")
                        bcb = bcp.tile([D, 2, NBLK], f32, tag="bcb",
                                       name="bcb")
                        rcb = bcp.tile([D, 2, NBLK], f32, tag="rcb",
                                       name="rcb")
                        scr = dscr.tile([2, NBLK], f32, tag="scr",
                                        name="scr")
                        nc.vector.tensor_copy(rawa[0:D + 1, 0:cw],
                                              oa[0:D + 1, 0:cw])
                        nc.vector.tensor_copy(rawb[0:D + 1, 0:cw],
                                              ob[0:D + 1, 0:cw])
                        for i, raw in ((0, rawa), (1, rawb)):
                            nc.sync.dma_start(out=scr[i:i + 1, 0:cw],
                                              in_=raw[D:D + 1, 0:cw])
                            src = scr[i:i + 1, 0:cw]
                            bsrc = bass.AP(tensor=src.tensor,
                                           offset=src.offset,
                                           ap=[[0, D]] + src.ap[1:])
                            nc.sync.dma_start(out=rcb[0:D, i, 0:cw],
                                              in_=bsrc)
                        nc.vector.reciprocal_approx_fast(
                            out=bcb[0:D, :, 0:cw], in_=rcb[0:D, :, 0:cw])
                        nc.vector.tensor_mul(
                            o_sb[0:D, p, nsl], rawa[0:D, 0:cw],
                            bcb[0:D, 0, 0:cw])
                        nc.vector.tensor_mul(
                            tb[0:D, 0:cw], rawb[0:D, 0:cw],
                            bcb[0:D, 1, 0:cw])
                        nc.sync.dma_start(out=o_sb[D:P, p, nsl],
                                          in_=tb[0:D, 0:cw])

                    if kind != "b":
                        # the deferred pvs and normalize must all pop
                        # before the next subblock's own first pv
                        # (which reuses the single-buffered oa/ob)
                        rem = sorted(pts)
                        carry = [(i // 2, (lambda tp=tp, pv=pv: pv(tp)))
                                 for i, tp in enumerate(rem)]
                        carry.append(((len(rem) + 1) // 2, normalize))
                    else:
                        # final half: latency-optimized drain. ACT (its
                        # exp stream just ended) reads only the
                        # denominator rows, the idle PE broadcasts them,
                        # the muls read the PV accumulators straight from
                        # PSUM, and pair 3 of the out-proj takes the
                        # second head half straight from tb
                        for tp in sorted(pts):
                            pv(tp)
                        tb = rawp.tile([D, NBLK], bf16, tag="tb",
                                       name="tb")
                        bcb = bcp.tile([D, 2, NBLK], f32, tag="bcb",
                                       name="bcb")
                        den = bcp.tile([P, 2, NBLK], bf16, tag="den")
                        nc.scalar.activation(out=den[D:D + 1, 0, 0:cw],
                                             in_=oa[D:D + 1, 0:cw], func=CPY)
                        nc.scalar.activation(out=den[D:D + 1, 1, 0:cw],
                                             in_=ob[D:D + 1, 0:cw], func=CPY)
                        bca = aux.tile([P, NBLK], f32, tag="aux",
                                       name="bca")
                        bcq = aux.tile([P, NBLK], f32, tag="aux",
                                       name="bcq")
                        nc.tensor.matmul(
                            bca[0:D, 0:cw], lhsT=ones_sb[D:D + 1, :],
                            rhs=den[D:D + 1, 0, 0:cw],
                            start=True, stop=True)
                        nc.tensor.matmul(
                            bcq[0:D, 0:cw], lhsT=ones_sb[D:D + 1, :],
                            rhs=den[D:D + 1, 1, 0:cw],
                            start=True, stop=True)
                        HB = cw // 2
                        nc.vector.reciprocal_approx_fast(
                            out=bcb[0:D, 0, 0:cw], in_=bca[0:D, 0:cw])
                        for hh in range(2):
                            hsl = slice(hh * HB, (hh + 1) * HB)
                            osl = slice(nb * NBLK + c0 + hh * HB,
                                        nb * NBLK + c0 + (hh + 1) * HB)
                            nc.vector.tensor_mul(
                                o_sb[0:D, 3, osl], oa[0:D, hsl],
                                bcb[0:D, 0, hsl])
                            if hh == 0:
                                nc.vector.reciprocal_approx_fast(
                                    out=bcb[0:D, 1, 0:cw],
                                    in_=bcq[0:D, 0:cw])
                            nc.vector.tensor_mul(
                                tb[0:D, hsl], ob[0:D, hsl],
                                bcb[0:D, 1, hsl])
                            final.append(
                                lambda nt0=4 + (c0 + hh * HB) // P, tb=tb:
                                fin_tail_pair(nt0, tb, c0))
                # drain: the final half's out-proj tail
                for f in final:
                    f()

    nc.compile()
    return nc


def get_nc(nmt=None):
    if nmt is None:
        nmt = _CACHE.get("last_nmt", M // P)
    if ("nc", nmt) not in _CACHE:
        _CACHE[("nc", nmt)] = _build_nc(nmt)
    _CACHE["last_nmt"] = nmt
    return _CACHE[("nc", nmt)]


def make_in_maps(x, context, mask, Wq, Wkv, Wo, bo):
    """CPU glue: shard, transpose, cast, and compact keys by mask."""
    bf = ml_dtypes.bfloat16
    Wk = np.ascontiguousarray(Wkv[:, :INNER]).astype(bf)
    Wv = np.ascontiguousarray(Wkv[:, INNER:]).astype(bf)
    Wq_b = np.ascontiguousarray(Wq).astype(bf)
    Wo_b = np.ascontiguousarray(Wo).astype(bf)
    bo_f = np.ascontiguousarray(bo, dtype=np.float32).reshape(1, QD)

    idxs = [np.where(mask[b])[0] for b in range(B)]
    maxc = max(1, max(len(i) for i in idxs))
    nmt = (maxc + P - 1) // P
    mpad = nmt * P

    in_maps = []
    for c in range(8):
        b, s = c // 2, c % 2
        idx = idxs[b]
        cnt = len(idx)
        ctx_c = np.zeros((mpad, QD), dtype=np.float32)
        ctx_c[:cnt] = context[b][idx]
        mb = np.full(mpad, MASK_NEG, dtype=np.float32)
        mb[:cnt] = 0.0
        xT = np.ascontiguousarray(
            x[b, s * NCORE:(s + 1) * NCORE, :].T).astype(bf)
        ctxT = np.ascontiguousarray(ctx_c.T).astype(bf)
        mbt = np.ascontiguousarray(mb.reshape(nmt, P).T)
        in_maps.append({
            "xT": xT.reshape(4, P, NCORE),
            "ctxT": ctxT.reshape(4, P, mpad),
            "wq": Wq_b.reshape(4, P, INNER),
            "wk": Wk.reshape(4, P, INNER),
            "wv": Wv.reshape(4, P, INNER),
            "wo": Wo_b.reshape(4, P, QD),
            "bo": bo_f, "mb": mbt,
        })
    return in_maps, nmt


def assemble(results):
    out = np.empty((B, N, QD), dtype=np.float32)
    for c in range(8):
        b, s = c // 2, c % 2
        out[b, s * NCORE:(s + 1) * NCORE, :] = np.asarray(
            results[c]["out"], dtype=np.float32)
    return out


def kernel(x, context, mask, Wq, Wkv, Wo, bo):
    from concourse.bass_utils import run_bass_kernel_spmd

    x = np.asarray(x, dtype=np.float32)
    context = np.asarray(context, dtype=np.float32)
    mask = np.asarray(mask)
    in_maps, nmt = make_in_maps(x, context, mask,
                                np.asarray(Wq, dtype=np.float32),
                                np.asarray(Wkv, dtype=np.float32),
                                np.asarray(Wo, dtype=np.float32),
                                np.asarray(bo, dtype=np.float32))
    nc = get_nc(nmt)
    res = run_bass_kernel_spmd(nc, in_maps, list(range(8)))
    return assemble(res.results)
